# revision 55
# baseline (speedup 1.0000x reference)
"""Bidirectional RoPE self-attention (Q is both query and key) on 8 trn2 cores.

Math (per (b,h) pair, T=1024, N=256):
    QR = rope(Q); S = QR @ QR.T / 16; out = softmax(S) @ V

Device strategy (fp8 end to end on the PE):
  - 96 (b,h) pairs sharded 12-per-core (batch/head parallel, no comm).
  - Host computes rope(Q) in fp32, scales by 1/4 (folds the 1/sqrt(256)
    softmax scale), casts to fp8e4m3 and pre-transposes to the DoubleRow
    [128, 2, T] channel-interleaved layout (elementwise O(TN) input prep,
    like the quantization itself; the DVE read-write-bubble errata makes
    on-device rope ~56us/core, and host rope also halves the Q DMA).
  - scores: one fp8 DoubleRow matmul per (t-tile, s-chunk): K=256 in a
    single pass. Scores land in fp32 PSUM [128, 1024] (2 banks).
  - exp: per-head constant bias -(max_t |Q_t|^2/16 - 10.4) keeps E in
    fp8e5m2 range (no overflow, relevant off-diag terms above the
    subnormal-flush zone) while PRESERVING the symmetry of E that the
    second matmul relies on. E tiles are written directly as fp8e5m2
    into [128, 2, T] chunk tiles pairing adjacent s-tiles for the
    DoubleRow moving layout.
  - exp is split across engines to balance them: chunks 0,2 of each pair
    go through a DVE Schraudolph trick (e5m2 bits are linear in log2, so
    uint8(round(5.77*x + B)) viewed as fp8e5 IS exp(x) to ~5%; softmax
    self-normalization absorbs most of it because numerator and
    denominator use the same bytes); chunks 1,3 go through the ScalarE
    Exp LUT (fp8e5 RNE out). Single writer engine per E tile.
  - Z: the softmax denominator must be the sum of the QUANTIZED E values
    actually fed to the matmul (activation accum_out taps pre-quant fp32
    - measured), so the fp8 E tiles are DMA'd to the host, which
    computes Z as column sums of exactly the bytes the matmul consumed
    and applies 1/Z. This also keeps normalization exact with two
    different quantizers in play.
  - attn @ V, transposed: E is symmetric, so E tiles [t, s] are also
    [s, t]; outT[n, t] = sum_s V8[s, n] E[s, t] with V8 as fp8e4m3
    DoubleRow stationary and E fp8e5m2 moving: both matmuls run at the
    fp8 2x rate (mixed e4m3/e5m2 operands measured exact).
  - V fp8 quantization is repaired on the host: out += dV[t] (dV = V-V8),
    exploiting diag-dominant attention (A_tt ~ 1); residual error
    <= 2*m*|dV| ~ 1e-2 absolute worst-case. Output DMAs as bf16.
  - schedule: pair-skewed pipeline. Slot p interleaves scores+exp for
    pair p with attn@V for pair p-1 (PE order S0 A0 S1 A1 S2 A2 S3 A3).
    Each attn GROUP (nch,tch) accumulates all 4 s-chunks back-to-back
    into one 1-bank PSUM tile then drains immediately (DVE or ScalarE,
    split to balance; separate staging tiles keep one writer per tile)
    and DMAs that [128,512] straight out. PSUM: 3x2 banks for scores +
    2x1 banks for attn output = all 8, double-buffered everywhere, so
    no write-after-read stalls on the PE. All per-pair DMA triggers live
    on the Sync engine: its queue has hardware descriptor generation,
    while GpSimd's is software-DGE (its final queue drain would gate the
    kernel epilogue), and ScalarE/DVE must do pure compute. Dummy
    matmuls at t=0 warm the PE clock gate (HAM) while the first loads
    are in flight.
"""

from contextlib import ExitStack

import numpy as np

import concourse.bacc as bacc
import concourse.tile as tile
from concourse import mybir

B, NH, T, N = 8, 12, 1024, 256
NCORES = 8
PAIRS = B * NH // NCORES  # 12 (b,h) pairs per core
F32 = mybir.dt.float32
BF16 = mybir.dt.bfloat16
FP8E4 = mybir.dt.float8e4
FP8E5 = mybir.dt.float8e5
U8 = mybir.dt.uint8
EXP = mybir.ActivationFunctionType.Exp
DR = mybir.MatmulPerfMode.DoubleRow

NTT = T // 128   # 8 t-tiles per pair
NCH = NTT // 2   # 4 DoubleRow s-chunks (K=256 each) for attn@V
SHIFT_MARGIN = 10.4
SCH_CHUNKS = (0, 2)  # chunks per pair through DVE Schraudolph (rest: ScalarE)
A8 = float(4.0 / np.log(2.0))
B8 = float(60.0 - 0.5 * 4 * 0.0861 / np.log(2.0))


def build_nc(pairs=PAIRS):
    nc = bacc.Bacc("TRN2", target_bir_lowering=False, debug=False,
                   enable_asserts=False)

    qr = nc.dram_tensor("qr", [pairs, 128, 2 * T], FP8E4, kind="ExternalInput")
    v = nc.dram_tensor("v", [pairs, 128, NCH * 2 * N], FP8E4,
                       kind="ExternalInput")
    db = nc.dram_tensor("db", [128, 2 * pairs], F32, kind="ExternalInput")
    # output split by draining engine so every DMA is a flat contiguous
    # 2D copy (strided/3D patterns force software descriptor generation
    # + engine queue drains)
    outt_a = nc.dram_tensor("outt_a", [pairs, 128, 512], BF16,
                            kind="ExternalOutput")
    outt_b = nc.dram_tensor("outt_b", [pairs, 128, 3 * 512], BF16,
                            kind="ExternalOutput")
    # chunk order in dram is [0, 2, 1, 3] (grouped by producing engine);
    # the host only ever sums E over all rows, so order is irrelevant
    edump = nc.dram_tensor("edump", [pairs, 2, 128, 2 * 2 * T], FP8E5,
                           kind="ExternalOutput")

    with tile.TileContext(nc) as tc, ExitStack() as ctx:
        cpool = ctx.enter_context(tc.tile_pool(name="cs", bufs=1))
        qrpool = ctx.enter_context(tc.tile_pool(name="qr", bufs=3))
        epool = ctx.enter_context(tc.tile_pool(name="e", bufs=2))
        vpool = ctx.enter_context(tc.tile_pool(name="v", bufs=3))
        opool = ctx.enter_context(tc.tile_pool(name="o", bufs=2))
        ps_s = ctx.enter_context(tc.tile_pool(name="ps_s", bufs=6, space="PSUM"))
        ps_o = ctx.enter_context(tc.tile_pool(name="ps_o", bufs=2, space="PSUM"))

        def load_pair(p):
            # flat 2D contiguous copies (the DoubleRow interleave is
            # baked into the host layout) keep the DMA descriptors
            # simple: no software descriptor generation, no engine
            # queue drains.
            qr8 = qrpool.tile([128, 2 * T], FP8E4)
            nc.sync.dma_start(qr8[:], qr[p])
            v8 = vpool.tile([128, NCH * 2 * N], FP8E4)
            nc.sync.dma_start(v8[:], v[p])
            return qr8, v8

        qrs, v8s, ets = {}, {}, {}
        qrs[0], v8s[0] = load_pair(0)

        dbt = cpool.tile([128, 2 * pairs], F32, tag="db")
        nc.scalar.dma_start(dbt[:], db[:])

        # warm the PE clock gate (HAM) with dummy matmuls while the first
        # qr/v DMAs are in flight; garbage values, result discarded
        warm = cpool.tile([128, 512], BF16, tag="warm")
        nc.vector.memset(warm[:], 0.0)
        wps = ps_o.tile([128, 512], F32, name="wps", tag="po")
        for _ in range(5):
            nc.tensor.matmul(wps[:], warm[:, 0:128], warm[:],
                             start=True, stop=True)

        # Pair-skewed pipeline: slot p emits scores+exp for pair p
        # interleaved with the four attn@V accumulation groups for pair
        # p-1 (each group = one (nch,tch) output tile accumulated over
        # all 4 s-chunks, drained and DMA'd out immediately).
        for p in range(pairs + 1):
            if p < pairs:
                if p + 1 < pairs:
                    qrs[p + 1], v8s[p + 1] = load_pair(p + 1)
                ets[p] = {}
                qr3 = qrs[p][:].rearrange("p (j t) -> p j t", j=2)
            if p < pairs:
                # two E staging tiles per pair, one per producing engine
                # (single writer each): eA = chunks 0,2 via DVE
                # Schraudolph, eB = chunks 1,3 via ScalarE Exp LUT.
                ets[p] = (
                    epool.tile([128, 2 * 2 * T], FP8E5, name="eA", tag="eA"),
                    epool.tile([128, 2 * 2 * T], FP8E5, name="eB", tag="eB"),
                )
            for c in range(4):
                if p < pairs:
                    # scores + exp for (p, c)
                    grp, idx = c % 2, c // 2
                    etile = ets[p][grp]
                    for j in range(2):
                        tt = 2 * c + j
                        # one 1-bank PSUM tile per 512-wide score slice;
                        # exp consumes each half right away so the six
                        # rotating banks never stall the PE on recycle
                        for sc in range(T // 512):
                            ps = ps_s.tile([128, 512], F32, name="ps")
                            nc.tensor.matmul(
                                ps[:],
                                qr3[:, :, tt * 128:(tt + 1) * 128],
                                qr3[:, :, sc * 512:(sc + 1) * 512],
                                start=True, stop=True, perf_mode=DR,
                            )
                            off = idx * 2 * T + j * T + sc * 512
                            dst = etile[:, off:off + 512]
                            if grp == 0:
                                # Schraudolph exp on DVE: e5m2 bits are
                                # linear in log2(E); fp32->uint8
                                # conversion rounds+saturates
                                nc.vector.tensor_scalar(
                                    dst.bitcast(U8), ps[:],
                                    A8, dbt[:, pairs + p:pairs + p + 1],
                                    mybir.AluOpType.mult,
                                    mybir.AluOpType.add)
                            else:
                                nc.scalar.activation(dst, ps[:], EXP,
                                                     bias=dbt[:, p:p + 1])
                    if c >= 2:
                        # E bytes to the host for the exact-Z column
                        # sums; one DMA per staging tile. Everything
                        # rides the Sync hardware-DGE queue: GpSimd's
                        # queue is software descriptor generation and
                        # its final drain would gate the epilogue.
                        nc.sync.dma_start(edump[p, grp], etile[:])

                if p >= 1:
                    # attn@V group g of pair p-1: accumulate all 4
                    # s-chunks into one PSUM bank, drain, DMA out.
                    # In the final attn-only slot, run the ScalarE-
                    # drained groups (1,2,3) first and the DVE-drained
                    # g0 last: the serial ScalarE copies and the big
                    # outt_b transfer then overlap the remaining
                    # matmuls instead of trailing the last one.
                    p2 = p - 1
                    g = (1, 2, 3, 0)[c] if p == pairs else c
                    nch, tch = divmod(g, 2)
                    v5 = v8s[p2][:].rearrange("p (c j n m) -> p c j n m",
                                              c=NCH, j=2, n=2)
                    po = ps_o.tile([128, 512], F32, name=f"po{g}", tag="po")
                    for cc in range(NCH):
                        et = ets[p2][cc % 2]
                        e3 = et[:].rearrange("p (i j t) -> p i j t",
                                             i=2, j=2)
                        nc.tensor.matmul(
                            po[:],
                            v5[:, cc, :, nch, :],
                            e3[:, cc // 2, :, tch * 512:(tch + 1) * 512],
                            start=(cc == 0), stop=(cc == NCH - 1),
                            perf_mode=DR,
                        )
                    # drain split keeps DVE/ScalarE balanced with the
                    # exp work; separate staging tiles per engine keep a
                    # single writer per tile; one outt DMA per staging
                    if g == 0:
                        o8a = opool.tile([128, 512], BF16, name="oga",
                                         tag="oga")
                        nc.vector.tensor_copy(o8a[:], po[:])
                        nc.sync.dma_start(outt_a[p2], o8a[:])
                    else:
                        if g == 1:
                            o8b = opool.tile([128, 3 * 512], BF16,
                                             name="ogb", tag="ogb")
                        nc.scalar.copy(
                            o8b[:, (g - 1) * 512:g * 512], po[:])
                        if g == 3:
                            nc.sync.dma_start(outt_b[p2], o8b[:])
                    if c == 3:
                        qrs.pop(p2), v8s.pop(p2), ets.pop(p2)

    nc.compile()
    return nc


def host_prep(Q, V, freqs):
    """Returns per-core in_maps for the 8 cores + host-side dV."""
    import ml_dtypes
    e4 = ml_dtypes.float8_e4m3

    Q = np.ascontiguousarray(np.asarray(Q), dtype=np.float32)
    V = np.ascontiguousarray(np.asarray(V), dtype=np.float32)
    freqs = np.asarray(freqs, dtype=np.float32)

    # rope on host (fp32), scaled by 1/4 so S lands in PSUM as S/16.
    half = freqs.reshape(-1)[0::2]  # [128] cycles-per-step
    t_col = np.arange(T, dtype=np.float32).reshape(T, 1)
    phases = t_col * half.reshape(1, 128)  # [T, 128] fp32
    ang = np.mod(phases, np.float32(1.0)) * np.float32(2.0 * np.pi)
    C = np.cos(ang).astype(np.float32) * np.float32(0.25)  # [T, 128]
    S = np.sin(ang).astype(np.float32) * np.float32(0.25)

    G = B * NH
    Qg = Q.reshape(G, T, N)
    q0 = Qg[:, :, 0::2]  # even channels [G, T, 128]
    q1 = Qg[:, :, 1::2]
    # QR in DoubleRow [128, 2, T] layout: slot 0 = even-channel rows,
    # slot 1 = odd-channel rows, both transposed to [n, t].
    QR = np.empty((G, 128, 2, T), e4)
    QR[:, :, 0] = (q0 * C - q1 * S).transpose(0, 2, 1).astype(e4)
    QR[:, :, 1] = (q1 * C + q0 * S).transpose(0, 2, 1).astype(e4)

    # per-head exp shift: max_t |Q_t|^2/16 - margin (rope preserves norms)
    dstar = np.einsum('gtn,gtn->gt', Qg, Qg, dtype=np.float64) / 16.0
    shift = (dstar.max(axis=1) - SHIFT_MARGIN).astype(np.float32)  # [G]

    # V8 fp8e4m3 in DoubleRow stationary layout [g, p, c, j, nch, m]
    # (s = 256c + 128j + p, n = 128nch + m); dV = V - V8 stays on host.
    Vg = V.reshape(G, T, N)
    V8 = Vg.astype(e4)
    dV = Vg - V8.astype(np.float32)
    V8l = np.ascontiguousarray(
        V8.reshape(G, NCH, 2, 128, 2, 128).transpose(0, 3, 1, 2, 4, 5))

    QRf = QR.reshape(G, 128, 2 * T)
    V8f = V8l.reshape(G, 128, NCH * 2 * N)
    in_maps = []
    for cidx in range(NCORES):
        sl = slice(cidx * PAIRS, (cidx + 1) * PAIRS)
        dbc = np.empty((128, 2 * PAIRS), np.float32)
        dbc[:, :PAIRS] = -shift[sl]                      # ScalarE Exp bias
        dbc[:, PAIRS:] = B8 - A8 * shift[sl]             # Schraudolph offset
        in_maps.append({"qr": QRf[sl], "v": V8f[sl], "db": dbc})
    return in_maps, dV


_CACHED_NC = None


def kernel(Q, V, freqs):
    global _CACHED_NC
    from concourse.bass_utils import run_bass_kernel_spmd

    in_maps, dV = host_prep(Q, V, freqs)
    if _CACHED_NC is None:
        _CACHED_NC = build_nc()
    res = run_bass_kernel_spmd(_CACHED_NC, in_maps, list(range(NCORES)))
    # outt_{a,b} concatenated: [pairs, 128 (n%128), 2*T (flat (n//128, t))]
    full = np.concatenate(
        [np.concatenate([res.results[c]["outt_a"].astype(np.float32),
                         res.results[c]["outt_b"].astype(np.float32)],
                        axis=-1) for c in range(NCORES)])
    full = full.reshape(B * NH, 128, 2, T).transpose(0, 3, 2, 1)
    full = full.reshape(B * NH, T, N)
    # Z[g, t] = sum over stored rows s of the exact fp8 bytes the matmul
    # used: edump [pairs, c, p, (j t)] with s = 256c + 128j + p
    ec = np.concatenate([res.results[c]["edump"] for c in range(NCORES)])
    ef = ec.reshape(B * NH, NCH, 128, 2, T).astype(np.float32)
    Z = ef.sum(axis=(1, 2, 3))
    out = full / Z[:, :, None] + dV
    return np.ascontiguousarray(out.astype(np.float32)).reshape(B, NH, T, N)
